# revision 1
# baseline (speedup 1.0000x reference)
import sys
sys.path.insert(0, '/opt/trn_rl_repo')
import numpy as np

N = 25000
E = 400000
NCORES = 8
NPC = 3200            # padded nodes per core (25 windows x 128)
NWIN = 25
TBL = 25600           # node table rows: 3200 own + 22400 others (padded)
ELEM = 192            # gather row: s1(64) | v1 c-major(96) | pad(32)

_CACHE = {}


def _prep_weights(W_sc_s, W_sc_v, W1_s, W1_v, W_r1, W_r2, W2_s, W2_v):
    c_s, c_x = np.sin(np.pi / 8.0), np.cos(np.pi / 8.0)
    Wnode = np.zeros((160, 256), np.float32)
    Wnode[0:64, 0:64] = W1_s / 8.0
    for c in range(3):
        Wnode[64 + 32 * c:96 + 32 * c, 64 + 32 * c:96 + 32 * c] = W1_v / np.sqrt(32.0)
    Wsc = np.zeros((160, 256), np.float32)
    Wsc[0:64, 0:96] = W_sc_s / 8.0 * c_s
    for c in range(3):
        Wsc[64 + 32 * c:96 + 32 * c, 96 + 32 * c:128 + 32 * c] = W_sc_v / np.sqrt(32.0) * c_s
    Wr1p = np.ascontiguousarray((W_r1 / np.sqrt(12.0)).astype(np.float32))
    # Wr2p columns: [w1(0:64) | w3rep(64:160) | w2(160:224) | w5rep(224:320) | w4rep(320:416)]
    Wr2p = np.zeros((100, 416), np.float32)
    Wr2p[:, 0:64] = W_r2[:, 0:64] / 10.0
    Wr2p[:, 160:224] = W_r2[:, 64:128] / 10.0
    for c in range(3):
        Wr2p[:, 64 + 32 * c:96 + 32 * c] = W_r2[:, 128:160] / 10.0
        Wr2p[:, 224 + 32 * c:256 + 32 * c] = W_r2[:, 192:224] / (10.0 * np.sqrt(2.0))
        Wr2p[:, 320 + 32 * c:352 + 32 * c] = W_r2[:, 160:192] / (10.0 * np.sqrt(3.0))
    # mid layout: [m0a 0:64 | m1b 64:160 | m0b 160:192 | m1a_x 192:256 | m1a_y | m1a_z |
    #              m1c_x 384:416 | m1c_y 416:448 | m1c_z 448:480]
    W2p = np.zeros((480, 256), np.float32)
    ks = c_x / np.sqrt(96.0) / 4.0
    kv = c_x / np.sqrt(128.0) / 4.0
    W2p[0:64, 0:96] = W2_s[0:64] * ks
    W2p[160:192, 0:96] = W2_s[64:96] * ks
    for c in range(3):
        W2p[64 + 32 * c:96 + 32 * c, 96 + 32 * c:128 + 32 * c] = W2_v[64:96] * kv
        W2p[192 + 64 * c:256 + 64 * c, 96 + 32 * c:128 + 32 * c] = W2_v[0:64] * kv
        W2p[384 + 32 * c:416 + 32 * c, 96 + 32 * c:128 + 32 * c] = W2_v[96:128] * kv
    return Wnode, Wsc, Wr1p, Wr2p, W2p


def _prep_core(c, x, edge_src, edge_dst, edge_attr, edge_scalars, WT):
    xrow = np.concatenate([np.arange(64), 64 + 3 * np.arange(32),
                           65 + 3 * np.arange(32), 66 + 3 * np.arange(32)])
    own0 = c * NPC
    own_n = min(NPC, N - own0)
    xp = np.zeros((TBL, 160), np.float32)
    xp[:own_n] = x[own0:own0 + own_n][:, xrow]
    other = np.concatenate([np.arange(0, own0), np.arange(own0 + own_n, N)])
    xp[NPC:NPC + other.size] = x[other][:, xrow]
    pos = np.empty(N, np.int64)
    pos[own0:own0 + own_n] = np.arange(own_n)
    pos[other] = NPC + np.arange(other.size)

    sel = np.nonzero((edge_dst >= own0) & (edge_dst < own0 + own_n))[0]
    dl = edge_dst[sel] - own0
    win = dl >> 7
    order = np.argsort(win, kind='stable')
    sel = sel[order]
    dl = dl[order]
    win = win[order]

    EP = NWIN * WT * 128
    es_p = np.zeros((EP, 12), np.float32)
    ea_p = np.zeros((EP, 4), np.float32)
    src_p = np.zeros(EP, np.int64)
    col_p = np.full(EP, -1.0, np.float32)
    for w in range(NWIN):
        m = win == w
        ew = sel[m]
        k = ew.size
        o = w * WT * 128
        es_p[o:o + k] = edge_scalars[ew]
        ea_p[o:o + k] = edge_attr[ew]
        src_p[o:o + k] = pos[edge_src[ew]]
        col_p[o:o + k] = (dl[m] & 127).astype(np.float32)

    T = EP // 128
    eaT = np.ascontiguousarray(ea_p.reshape(T, 128, 4).transpose(1, 0, 2).reshape(128, T * 4))
    dstT = np.ascontiguousarray(col_p.reshape(T, 128).T)
    esT = np.ascontiguousarray(es_p.T)
    idx16 = src_p.astype(np.int16).reshape(-1, 16).T       # [16, EP/16]
    srcIdx = np.ascontiguousarray(np.tile(idx16, (8, 1)))  # [128, EP/16]
    return dict(xT=np.ascontiguousarray(xp.T), esT=esT, eaT=eaT, dstT=dstT,
                srcIdx=srcIdx)


def _build_program(WT):
    import concourse.bass as bass
    import concourse.tile as tile
    from concourse import bacc, mybir

    f32 = mybir.dt.float32
    f32r = mybir.dt.float32r
    i16 = mybir.dt.int16
    i32 = mybir.dt.int32
    AF = mybir.ActivationFunctionType
    EP = NWIN * WT * 128

    nc = bacc.Bacc("TRN2", num_devices=NCORES, debug=False)
    xT_ap = nc.dram_tensor("xT", [160, TBL], f32r, kind="ExternalInput").ap()
    esT_ap = nc.dram_tensor("esT", [12, EP], f32r, kind="ExternalInput").ap()
    eaT_ap = nc.dram_tensor("eaT", [128, (EP // 128) * 4], f32, kind="ExternalInput").ap()
    dstT_ap = nc.dram_tensor("dstT", [128, EP // 128], f32, kind="ExternalInput").ap()
    idx_ap = nc.dram_tensor("srcIdx", [128, EP // 16], i16, kind="ExternalInput").ap()
    Wnode_ap = nc.dram_tensor("Wnode", [160, 256], f32r, kind="ExternalInput").ap()
    Wsc_ap = nc.dram_tensor("Wsc", [160, 256], f32r, kind="ExternalInput").ap()
    Wr1_ap = nc.dram_tensor("Wr1p", [12, 100], f32r, kind="ExternalInput").ap()
    Wr2_ap = nc.dram_tensor("Wr2p", [100, 416], f32r, kind="ExternalInput").ap()
    W2p_ap = nc.dram_tensor("W2p", [480, 256], f32r, kind="ExternalInput").ap()
    out_ap = nc.dram_tensor("out", [NPC, 160], f32, kind="ExternalOutput").ap()

    with tile.TileContext(nc) as tc:
        from contextlib import ExitStack
        with ExitStack() as ctx:
            wpool = ctx.enter_context(tc.tile_pool(name="weights", bufs=1))
            dram = ctx.enter_context(tc.tile_pool(name="ndram", bufs=1, space="DRAM"))
            ntab = dram.tile([TBL, ELEM], f32)

            wn1 = wpool.tile([128, 256], f32r)
            wn2 = wpool.tile([32, 256], f32r)
            ws1 = wpool.tile([128, 256], f32r)
            ws2 = wpool.tile([32, 256], f32r)
            wr1 = wpool.tile([12, 100], f32r)
            wr2 = wpool.tile([100, 416], f32r)
            w2p = [wpool.tile([120, 256], f32r, tag=f"w2p{j}", name=f"w2p{j}")
                   for j in range(4)]
            nc.sync.dma_start(wn1[:], Wnode_ap[0:128, :])
            nc.sync.dma_start(wn2[:], Wnode_ap[128:160, :])
            nc.sync.dma_start(ws1[:], Wsc_ap[0:128, :])
            nc.sync.dma_start(ws2[:], Wsc_ap[128:160, :])
            nc.sync.dma_start(wr1[:], Wr1_ap[:])
            nc.sync.dma_start(wr2[:], Wr2_ap[:])
            for j in range(4):
                nc.sync.dma_start(w2p[j][:], W2p_ap[j * 120:(j + 1) * 120, :])

            ioti = wpool.tile([128, 128], i32)
            iotf = wpool.tile([128, 128], f32)
            iotci = wpool.tile([128, 1], i32)
            iotcf = wpool.tile([128, 1], f32)
            ident = wpool.tile([128, 128], f32)
            nc.gpsimd.iota(ioti[:], pattern=[[1, 128]], base=0, channel_multiplier=0)
            nc.vector.tensor_copy(iotf[:], ioti[:])
            nc.gpsimd.iota(iotci[:], pattern=[[0, 1]], base=0, channel_multiplier=1)
            nc.vector.tensor_copy(iotcf[:], iotci[:])
            nc.vector.tensor_scalar(ident[:], iotf[:], iotcf[:], None,
                                    op0=mybir.AluOpType.is_equal)
            scN = wpool.tile([128, NWIN * 192], f32)

            # Phase A: node table (lin1) + self-connection
            with tc.tile_pool(name="xa", bufs=3) as xa, \
                 tc.tile_pool(name="xb", bufs=3) as xb, \
                 tc.tile_pool(name="ntp", bufs=2, space="PSUM") as ntp, \
                 tc.tile_pool(name="scp", bufs=2, space="PSUM") as scp, \
                 tc.tile_pool(name="nts", bufs=3) as ntsp:
                for b in range(TBL // 128):
                    xc1 = xa.tile([128, 128], f32r)
                    xc2 = xb.tile([32, 128], f32r)
                    nc.sync.dma_start(xc1[:], xT_ap[0:128, b * 128:(b + 1) * 128])
                    nc.sync.dma_start(xc2[:], xT_ap[128:160, b * 128:(b + 1) * 128])
                    pt = ntp.tile([128, 256], f32)
                    nc.tensor.matmul(pt[:], xc1[:], wn1[:],
                                     start=True, stop=False)
                    nc.tensor.matmul(pt[:], xc2[:], wn2[:],
                                     start=False, stop=True)
                    nt = ntsp.tile([128, ELEM], f32)
                    nc.scalar.activation(nt[:], pt[:, 0:ELEM], AF.Copy)
                    nc.sync.dma_start(ntab[b * 128:(b + 1) * 128, :], nt[:])
                    if b < NWIN:
                        st = scp.tile([128, 256], f32)
                        nc.tensor.matmul(st[:], xc1[:], ws1[:],
                                         start=True, stop=False)
                        nc.tensor.matmul(st[:], xc2[:], ws2[:],
                                         start=False, stop=True)
                        nc.scalar.activation(scN[:, b * 192:(b + 1) * 192], st[:, 0:192],
                                             AF.Copy)

            # Phase B: edges
            if True:
                esP = ctx.enter_context(tc.tile_pool(name="esw", bufs=2))
                eaP = ctx.enter_context(tc.tile_pool(name="eaw", bufs=2))
                dsP = ctx.enter_context(tc.tile_pool(name="dsw", bufs=2))
                idP = ctx.enter_context(tc.tile_pool(name="idxw", bufs=2))
                gP = ctx.enter_context(tc.tile_pool(name="gat", bufs=2))
                hP = ctx.enter_context(tc.tile_pool(name="hp", bufs=2, space="PSUM"))
                hsP = ctx.enter_context(tc.tile_pool(name="hs", bufs=2))
                wpP = ctx.enter_context(tc.tile_pool(name="wp", bufs=2, space="PSUM"))
                wsP = ctx.enter_context(tc.tile_pool(name="wsb", bufs=2))
                pP = ctx.enter_context(tc.tile_pool(name="pp", bufs=2))
                mP = ctx.enter_context(tc.tile_pool(name="mid", bufs=2))
                tP = ctx.enter_context(tc.tile_pool(name="tmp", bufs=2))
                ohP = ctx.enter_context(tc.tile_pool(name="oh", bufs=2))
                accP = ctx.enter_context(tc.tile_pool(name="acc", bufs=2, space="PSUM"))
                tlP = ctx.enter_context(tc.tile_pool(name="tail", bufs=2))
                tpsP = ctx.enter_context(tc.tile_pool(name="tps", bufs=1, space="PSUM"))
                ypP = ctx.enter_context(tc.tile_pool(name="yp", bufs=1, space="PSUM"))
                oP = ctx.enter_context(tc.tile_pool(name="outs", bufs=2))
                TW = WT * 128
                for w in range(NWIN):
                    esw = esP.tile([12, TW], f32r)
                    nc.sync.dma_start(esw[:], esT_ap[:, w * TW:(w + 1) * TW])
                    eaw = eaP.tile([128, 4 * WT], f32)
                    nc.sync.dma_start(eaw[:], eaT_ap[:, w * 4 * WT:(w + 1) * 4 * WT])
                    dsw = dsP.tile([128, WT], f32)
                    nc.sync.dma_start(dsw[:], dstT_ap[:, w * WT:(w + 1) * WT])
                    idxw = idP.tile([128, 8 * WT], i16)
                    nc.sync.dma_start(idxw[:], idx_ap[:, w * 8 * WT:(w + 1) * 8 * WT])
                    gt = gP.tile([128, WT, ELEM], f32)
                    nc.gpsimd.dma_gather(gt[:], ntab[:], idxw[:], TW, TW, ELEM,
                                         single_packet=False)

                    hsb = hsP.tile([100, TW], f32r)
                    for j in range(TW // 384):
                        hp = hP.tile([100, 384], f32)
                        nc.tensor.matmul(hp[:], wr1[:],
                                         esw[:, j * 384:(j + 1) * 384],
                                         start=True, stop=True)
                        hsg = hsP.tile([100, 384], f32, tag="hsg")
                        nc.scalar.activation(hsg[:], hp[:], AF.Sigmoid)
                        nc.vector.tensor_mul(hsb[:, j * 384:(j + 1) * 384], hsg[:], hp[:])

                    acc = accP.tile([128, 480], f32)
                    for t in range(WT):
                        wpp = wpP.tile([128, 416], f32)
                        nc.tensor.matmul(wpp[:], hsb[:, t * 128:(t + 1) * 128],
                                         wr2[:], start=True, stop=True)
                        wsb = wsP.tile([128, 416], f32)
                        nc.scalar.activation(wsb[:], wpp[:], AF.Copy)
                        g = gt[:, t, :]
                        P = pP.tile([128, 416], f32)
                        nc.vector.tensor_mul(P[:, 0:160], wsb[:, 0:160], g[:, 0:160])
                        nc.vector.tensor_mul(P[:, 160:224], wsb[:, 160:224], g[:, 0:64])
                        nc.vector.tensor_mul(P[:, 224:320], wsb[:, 224:320], g[:, 64:160])
                        nc.vector.tensor_mul(P[:, 320:416], wsb[:, 320:416], g[:, 64:160])
                        se = eaw[:, 4 * t + 0:4 * t + 1]
                        vx = eaw[:, 4 * t + 1:4 * t + 2]
                        vy = eaw[:, 4 * t + 2:4 * t + 3]
                        vz = eaw[:, 4 * t + 3:4 * t + 4]
                        mid = mP.tile([128, 480], f32r)
                        tmp = tP.tile([128, 160], f32)
                        nc.scalar.activation(mid[:, 0:160], P[:, 0:160], AF.Copy, scale=se)
                        # m0b = sum_c (w4 vv_c) ve_c
                        nc.vector.tensor_scalar_mul(mid[:, 160:192], P[:, 320:352], vx)
                        nc.vector.tensor_scalar_mul(tmp[:, 0:32], P[:, 352:384], vy)
                        nc.vector.tensor_scalar_mul(tmp[:, 32:64], P[:, 384:416], vz)
                        nc.vector.tensor_add(mid[:, 160:192], mid[:, 160:192], tmp[:, 0:32])
                        nc.vector.tensor_add(mid[:, 160:192], mid[:, 160:192], tmp[:, 32:64])
                        # m1a_c = (w2 ss) ve_c
                        nc.vector.tensor_scalar_mul(mid[:, 192:256], P[:, 160:224], vx)
                        nc.vector.tensor_scalar_mul(mid[:, 256:320], P[:, 160:224], vy)
                        nc.vector.tensor_scalar_mul(mid[:, 320:384], P[:, 160:224], vz)
                        # m1c: P5x=[224:256] P5y=[256:288] P5z=[288:320]
                        nc.vector.tensor_scalar_mul(mid[:, 384:416], P[:, 256:288], vz)
                        nc.vector.tensor_scalar_mul(tmp[:, 64:96], P[:, 288:320], vy)
                        nc.vector.tensor_scalar_mul(mid[:, 416:448], P[:, 288:320], vx)
                        nc.vector.tensor_scalar_mul(tmp[:, 96:128], P[:, 224:256], vz)
                        nc.vector.tensor_scalar_mul(mid[:, 448:480], P[:, 224:256], vy)
                        nc.vector.tensor_scalar_mul(tmp[:, 128:160], P[:, 256:288], vx)
                        nc.vector.tensor_sub(mid[:, 384:416], mid[:, 384:416], tmp[:, 64:96])
                        nc.vector.tensor_sub(mid[:, 416:448], mid[:, 416:448], tmp[:, 96:128])
                        nc.vector.tensor_sub(mid[:, 448:480], mid[:, 448:480], tmp[:, 128:160])
                        oh = ohP.tile([128, 128], f32r)
                        nc.vector.tensor_scalar(oh[:], iotf[:], dsw[:, t:t + 1], None,
                                                op0=mybir.AluOpType.is_equal)
                        nc.tensor.matmul(acc[:], oh[:], mid[:],
                                         start=(t == 0), stop=(t == WT - 1))

                    # window tail: lin2 + sc + gate
                    asb = tlP.tile([128, 480], f32, tag="asb")
                    nc.scalar.activation(asb[:], acc[:], AF.Copy)
                    yp = ypP.tile([128, 256], f32)
                    for j in range(4):
                        tp = tpsP.tile([120, 128], f32, tag=f"tp")
                        nc.tensor.transpose(tp[:], asb[:, j * 120:(j + 1) * 120], ident[:])
                        ts = tlP.tile([120, 128], f32r, tag="ts")
                        nc.scalar.activation(ts[:], tp[:], AF.Copy)
                        nc.tensor.matmul(yp[:], ts[:], w2p[j][:],
                                         start=(j == 0), stop=(j == 3))
                    y1 = tlP.tile([128, 192], f32, tag="y1")
                    nc.scalar.activation(y1[:], yp[:, 0:192], AF.Copy)
                    y2 = tlP.tile([128, 192], f32, tag="y2")
                    nc.vector.tensor_add(y2[:], y1[:], scN[:, w * 192:(w + 1) * 192])
                    outt = oP.tile([128, 160], f32, tag="outt")
                    gtl = oP.tile([128, 32], f32, tag="gtl")
                    sgo = oP.tile([128, 64], f32, tag="sgo")
                    nc.scalar.activation(sgo[:], y2[:, 0:64], AF.Sigmoid)
                    nc.vector.tensor_mul(outt[:, 0:64], y2[:, 0:64], sgo[:])
                    nc.scalar.activation(gtl[:], y2[:, 64:96], AF.Sigmoid)
                    for c in range(3):
                        nc.vector.tensor_mul(outt[:, 64 + 32 * c:96 + 32 * c],
                                             y2[:, 96 + 32 * c:128 + 32 * c], gtl[:])
                    nc.sync.dma_start(out_ap[w * 128:(w + 1) * 128, :], outt[:])

    nc.compile()
    return nc


def kernel(x, z, edge_src, edge_dst, edge_attr, edge_scalars,
           W_sc_s, W_sc_v, W1_s, W1_v, W_r1, W_r2, W2_s, W2_v):
    from concourse import bass_utils
    x = np.asarray(x, np.float32)
    edge_src = np.asarray(edge_src, np.int64)
    edge_dst = np.asarray(edge_dst, np.int64)
    edge_attr = np.asarray(edge_attr, np.float32)
    edge_scalars = np.asarray(edge_scalars, np.float32)

    # uniform tiles-per-window across all cores/windows (SPMD: one program)
    counts = np.zeros((NCORES, NWIN), np.int64)
    cw = (edge_dst // NPC) * NWIN + (edge_dst % NPC) // 128
    u, ct = np.unique(cw, return_counts=True)
    counts.flat[u] = ct
    WT = int(np.ceil(counts.max() / 128.0))
    WT = ((WT + 2) // 3) * 3  # multiple of 3 for 384-wide radial matmuls

    key = WT
    if key not in _CACHE:
        _CACHE[key] = _build_program(WT)
    nc = _CACHE[key]

    Wnode, Wsc, Wr1p, Wr2p, W2p = _prep_weights(
        np.asarray(W_sc_s, np.float32), np.asarray(W_sc_v, np.float32),
        np.asarray(W1_s, np.float32), np.asarray(W1_v, np.float32),
        np.asarray(W_r1, np.float32), np.asarray(W_r2, np.float32),
        np.asarray(W2_s, np.float32), np.asarray(W2_v, np.float32))

    in_maps = []
    for c in range(NCORES):
        m = _prep_core(c, x, edge_src, edge_dst, edge_attr, edge_scalars, WT)
        m.update(Wnode=Wnode, Wsc=Wsc, Wr1p=Wr1p, Wr2p=Wr2p, W2p=W2p)
        in_maps.append(m)

    res = bass_utils.run_bass_kernel_spmd(nc, in_maps, core_ids=list(range(NCORES)))
    parts = []
    for c in range(NCORES):
        own_n = min(NPC, N - c * NPC)
        parts.append(res.results[c]["out"][:own_n])
    full = np.concatenate(parts, axis=0)
    out = np.empty((N, 160), np.float32)
    out[:, 0:64] = full[:, 0:64]
    # device gated layout is c-major [32c+u]; reference wants u-major [3u+c]
    out[:, 64:160] = full[:, 64:160].reshape(N, 3, 32).transpose(0, 2, 1).reshape(N, 96)
    return out



# revision 11
# speedup vs baseline: 3.1201x; 3.1201x over previous
import sys
sys.path.insert(0, '/opt/trn_rl_repo')
import numpy as np
import ml_dtypes

BF = ml_dtypes.bfloat16

N = 25000
E = 400000
NCORES = 8
NPC = 3200            # padded nodes per core (25 windows x 128)
NWIN = 25
TBL = 25600           # node table rows: 3200 own + 22400 others (padded)
GEL = 256             # gather row cols (bf16): ss(64) | vv(96) | pad(96); 512B
PW = 640              # P / wpp cols: [A 160 | P2|XB 160 | P2|YB 160 | P2|ZB 160]

_CACHE = {}


def _prep_weights(W_sc_s, W_sc_v, W1_s, W1_v, W_r1, W_r2, W2_s, W2_v):
    c_s, c_x = np.sin(np.pi / 8.0), np.cos(np.pi / 8.0)
    # lin1 -> gather-row layout [s1(64) | v1 c-major(96) | pad(96)]
    Wnode = np.zeros((160, GEL), np.float32)
    Wnode[0:64, 0:64] = W1_s / 8.0
    for c in range(3):
        Wnode[64 + 32 * c:96 + 32 * c, 64 + 32 * c:96 + 32 * c] = W1_v / np.sqrt(32.0)
    # self-connection -> scN layout [sc_s(96) | sc_v c-major(96)]
    Wsc = np.zeros((160, 192), np.float32)
    Wsc[0:64, 0:96] = W_sc_s / 8.0 * c_s
    for c in range(3):
        Wsc[64 + 32 * c:96 + 32 * c, 96 + 32 * c:128 + 32 * c] = \
            W_sc_v / np.sqrt(32.0) * c_s
    Wr1p = (W_r1 / np.sqrt(12.0)).astype(np.float32)
    # radial -> P col layout (640):
    #  [0:64]   w1          (A: m0a, via oh_se)
    #  [64:160] w3 rep x3   (A: m1b c-major, via oh_se)
    #  [160:224] w2  | [224:256] w4' | [256:288] -w5' | [288:320] +w5'   (R_x)
    #  [320:384] w2  | [384:416] +w5' | [416:448] w4' | [448:480] -w5'   (R_y)
    #  [480:544] w2  | [544:576] -w5' | [576:608] +w5' | [608:640] w4'   (R_z)
    w1 = W_r2[:, 0:64] / 10.0
    w2 = W_r2[:, 64:128] / 10.0
    w3 = W_r2[:, 128:160] / 10.0
    w4 = W_r2[:, 160:192] / (10.0 * np.sqrt(3.0))
    w5 = W_r2[:, 192:224] / (10.0 * np.sqrt(2.0))
    Wr2p = np.zeros((100, PW), np.float32)
    Wr2p[:, 0:64] = w1
    for c in range(3):
        Wr2p[:, 64 + 32 * c:96 + 32 * c] = w3
    for r, (ca, cb, cc) in zip((160, 320, 480),
                               (((w4, 1), (w5, -1), (w5, 1)),
                                ((w5, 1), (w4, 1), (w5, -1)),
                                ((w5, -1), (w5, 1), (w4, 1)))):
        Wr2p[:, r:r + 64] = w2
        Wr2p[:, r + 64:r + 96] = ca[0] * ca[1]
        Wr2p[:, r + 96:r + 128] = cb[0] * cb[1]
        Wr2p[:, r + 128:r + 160] = cc[0] * cc[1]
    # lin2: acc (640) -> y (192: [scal 64 | gates 32 | gated c-major 96])
    # acc layout:
    #  [0:64] m0a | [64:160] m1b c-major                       (R_se)
    #  [160:224] m1a_x | [224:256] m0b_x | [256:288] m1cz_x | [288:320] m1cy_x
    #  [320:384] m1a_y | [384:416] m1cz_y | [416:448] m0b_y | [448:480] m1cx_y
    #  [480:544] m1a_z | [544:576] m1cy_z | [576:608] m1cx_z | [608:640] m0b_z
    ks = c_x / np.sqrt(96.0) / 4.0
    kv = c_x / np.sqrt(128.0) / 4.0
    W2p = np.zeros((PW, 192), np.float32)
    W2p[0:64, 0:96] = W2_s[0:64] * ks
    for c in range(3):
        W2p[64 + 32 * c:96 + 32 * c, 96 + 32 * c:128 + 32 * c] = W2_v[64:96] * kv
    W2v0 = W2_v[0:64] * kv
    W2vc = W2_v[96:128] * kv
    W2sb = W2_s[64:96] * ks
    # R_x
    W2p[160:224, 96:128] = W2v0
    W2p[224:256, 0:96] = W2sb
    W2p[256:288, 160:192] = W2vc     # m1c_z
    W2p[288:320, 128:160] = W2vc     # m1c_y
    # R_y
    W2p[320:384, 128:160] = W2v0
    W2p[384:416, 160:192] = W2vc     # m1c_z
    W2p[416:448, 0:96] = W2sb
    W2p[448:480, 96:128] = W2vc      # m1c_x
    # R_z
    W2p[480:544, 160:192] = W2v0
    W2p[544:576, 128:160] = W2vc     # m1c_y
    W2p[576:608, 96:128] = W2vc      # m1c_x
    W2p[608:640, 0:96] = W2sb
    return (Wnode.astype(BF), Wsc.astype(BF), Wr1p.astype(BF),
            Wr2p.astype(BF), W2p.astype(BF))


def _prep_core(c, x, edge_src, edge_dst, edge_attr, edge_scalars, WT):
    xrow = np.concatenate([np.arange(64), 64 + 3 * np.arange(32),
                           65 + 3 * np.arange(32), 66 + 3 * np.arange(32)])
    own0 = c * NPC
    own_n = min(NPC, N - own0)
    xp = np.zeros((TBL, 160), np.float32)
    xp[:own_n] = x[own0:own0 + own_n][:, xrow]
    other = np.concatenate([np.arange(0, own0), np.arange(own0 + own_n, N)])
    xp[NPC:NPC + other.size] = x[other][:, xrow]
    pos = np.empty(N, np.int64)
    pos[own0:own0 + own_n] = np.arange(own_n)
    pos[other] = NPC + np.arange(other.size)

    sel = np.nonzero((edge_dst >= own0) & (edge_dst < own0 + own_n))[0]
    dl = edge_dst[sel] - own0
    win = dl >> 7
    src_pos = pos[edge_src[sel]]
    order = np.lexsort((src_pos, win))   # by window, then by src for locality
    sel = sel[order]
    dl = dl[order]
    win = win[order]
    src_pos = src_pos[order]

    EP = NWIN * WT * 128
    es_p = np.zeros((EP, 12), np.float32)
    src_p = np.zeros(EP, np.int64)
    slot_t = np.zeros(EP, np.int64)      # global tile index of each slot
    slot_p = np.zeros(EP, np.int64)
    slot_d = np.full(EP, -1, np.int64)
    ea_v = np.zeros((EP, 4), np.float32)
    for w in range(NWIN):
        m = win == w
        ew = sel[m]
        k = ew.size
        o = w * WT * 128
        es_p[o:o + k] = edge_scalars[ew]
        src_p[o:o + k] = src_pos[m]
        slot_d[o:o + k] = dl[m] & 127
        ea_v[o:o + k] = edge_attr[ew]
    sl = np.arange(EP)
    slot_t = sl >> 7
    slot_p = sl & 127

    T = EP // 128
    # oh4[t, g, p, d]: g order = (x, y, z, se) -> edge_attr cols (1, 2, 3, 0)
    oh4 = np.zeros((T, 4, 128, 128), np.float32)
    v = slot_d >= 0
    oh4[slot_t[v], :, slot_p[v], slot_d[v]] = ea_v[v][:, [1, 2, 3, 0]]
    oh4T = np.ascontiguousarray(
        oh4.transpose(2, 0, 1, 3).reshape(128, T * 512)).astype(BF)

    esT = np.ascontiguousarray(es_p.T).astype(BF)
    idx16 = src_p.astype(np.int16).reshape(-1, 16).T       # [16, EP/16]
    srcIdx = np.ascontiguousarray(np.tile(idx16, (8, 1)))  # [128, EP/16]
    return dict(xT=np.ascontiguousarray(xp.T).astype(BF), esT=esT,
                oh4=oh4T, srcIdx=srcIdx)


def _build_program(WT):
    import concourse.bass as bass
    import concourse.tile as tile
    from concourse import bacc, mybir

    f32 = mybir.dt.float32
    bf16 = mybir.dt.bfloat16
    i16 = mybir.dt.int16
    AF = mybir.ActivationFunctionType
    MUL = mybir.AluOpType.mult
    EP = NWIN * WT * 128

    nc = bacc.Bacc("TRN2", num_devices=NCORES, debug=False)
    xT_ap = nc.dram_tensor("xT", [160, TBL], bf16, kind="ExternalInput").ap()
    esT_ap = nc.dram_tensor("esT", [12, EP], bf16, kind="ExternalInput").ap()
    oh4_ap = nc.dram_tensor("oh4", [128, (EP // 128) * 512], bf16,
                            kind="ExternalInput").ap()
    idx_ap = nc.dram_tensor("srcIdx", [128, EP // 16], i16, kind="ExternalInput").ap()
    Wnode_ap = nc.dram_tensor("Wnode", [160, GEL], bf16, kind="ExternalInput").ap()
    Wsc_ap = nc.dram_tensor("Wsc", [160, 192], bf16, kind="ExternalInput").ap()
    Wr1_ap = nc.dram_tensor("Wr1p", [12, 100], bf16, kind="ExternalInput").ap()
    Wr2_ap = nc.dram_tensor("Wr2p", [100, PW], bf16, kind="ExternalInput").ap()
    W2p_ap = nc.dram_tensor("W2p", [PW, 192], bf16, kind="ExternalInput").ap()
    out_ap = nc.dram_tensor("out", [NPC, 160], f32, kind="ExternalOutput").ap()

    with tile.TileContext(nc) as tc:
        from contextlib import ExitStack
        with ExitStack() as ctx:
            wpool = ctx.enter_context(tc.tile_pool(name="weights", bufs=1))
            dram = ctx.enter_context(tc.tile_pool(name="ndram", bufs=1, space="DRAM"))
            ntab = dram.tile([TBL, GEL], bf16)

            wn1 = wpool.tile([128, GEL], bf16)
            wn2 = wpool.tile([32, GEL], bf16)
            ws1 = wpool.tile([128, 192], bf16)
            ws2 = wpool.tile([32, 192], bf16)
            wr1 = wpool.tile([12, 100], bf16)
            wr2 = wpool.tile([100, PW], bf16)
            w2p = [wpool.tile([128, 192], bf16, tag=f"w2p{j}", name=f"w2p{j}")
                   for j in range(5)]
            zt = wpool.tile([128, PW], bf16)
            ident = wpool.tile([128, 128], bf16)
            ioti = wpool.tile([128, 128], mybir.dt.int32)
            iotf = wpool.tile([128, 128], f32)
            iotci = wpool.tile([128, 1], mybir.dt.int32)
            iotcf = wpool.tile([128, 1], f32)
            nc.sync.dma_start(wn1[:], Wnode_ap[0:128, :])
            nc.sync.dma_start(wn2[:], Wnode_ap[128:160, :])
            nc.sync.dma_start(ws1[:], Wsc_ap[0:128, :])
            nc.sync.dma_start(ws2[:], Wsc_ap[128:160, :])
            nc.sync.dma_start(wr1[:], Wr1_ap[:])
            nc.sync.dma_start(wr2[:], Wr2_ap[:])
            for j in range(5):
                nc.sync.dma_start(w2p[j][:], W2p_ap[j * 128:(j + 1) * 128, :])
            nc.vector.memset(zt[:], 0.0)
            nc.gpsimd.iota(ioti[:], pattern=[[1, 128]], base=0, channel_multiplier=0)
            nc.vector.tensor_copy(iotf[:], ioti[:])
            nc.gpsimd.iota(iotci[:], pattern=[[0, 1]], base=0, channel_multiplier=1)
            nc.vector.tensor_copy(iotcf[:], iotci[:])
            nc.vector.tensor_scalar(ident[:], iotf[:], iotcf[:], None,
                                    op0=mybir.AluOpType.is_equal)
            scN = wpool.tile([128, NWIN * 192], bf16)

            # Phase A: node table (lin1 -> gather rows) + self-connection
            with tc.tile_pool(name="xa", bufs=3) as xa, \
                 tc.tile_pool(name="xb", bufs=3) as xb, \
                 tc.tile_pool(name="ntp", bufs=3, space="PSUM") as ntp, \
                 tc.tile_pool(name="scp", bufs=2, space="PSUM") as scp, \
                 tc.tile_pool(name="nts", bufs=3) as ntsp:
                for bo in range(TBL // 512):
                    xc1 = xa.tile([128, 512], bf16)
                    xc2 = xb.tile([32, 512], bf16)
                    nc.sync.dma_start(xc1[:], xT_ap[0:128, bo * 512:(bo + 1) * 512])
                    nc.sync.dma_start(xc2[:], xT_ap[128:160, bo * 512:(bo + 1) * 512])
                    nt = ntsp.tile([128, 4, GEL], bf16)
                    for j in range(4):
                        b = bo * 4 + j
                        pt = ntp.tile([128, GEL], f32)
                        nc.tensor.matmul(pt[:], xc1[:, j * 128:(j + 1) * 128],
                                         wn1[:], start=True, stop=False)
                        nc.tensor.matmul(pt[:], xc2[:, j * 128:(j + 1) * 128],
                                         wn2[:], start=False, stop=True)
                        nc.scalar.activation(nt[:, j, :], pt[:], AF.Copy)
                        if b < NWIN:
                            st = scp.tile([128, 192], f32)
                            nc.tensor.matmul(st[:], xc1[:, j * 128:(j + 1) * 128],
                                             ws1[:], start=True, stop=False)
                            nc.tensor.matmul(st[:], xc2[:, j * 128:(j + 1) * 128],
                                             ws2[:], start=False, stop=True)
                            nc.scalar.activation(scN[:, b * 192:(b + 1) * 192],
                                                 st[:], AF.Copy)
                    dst = ntab[bo * 512:(bo + 1) * 512, :].rearrange(
                        "(a p) b -> p a b", a=4)
                    nc.sync.dma_start(dst, nt[:])

            # Phase B: edges
            esP = ctx.enter_context(tc.tile_pool(name="esw", bufs=2))
            idP = ctx.enter_context(tc.tile_pool(name="idxw", bufs=2))
            ohP = ctx.enter_context(tc.tile_pool(name="ohw", bufs=2))
            gP = ctx.enter_context(tc.tile_pool(name="gat", bufs=2))
            hsP = ctx.enter_context(tc.tile_pool(name="hs", bufs=2))
            wpP = ctx.enter_context(tc.tile_pool(name="wp", bufs=2, space="PSUM"))
            pP = ctx.enter_context(tc.tile_pool(name="pp", bufs=2))
            accP = ctx.enter_context(tc.tile_pool(name="acc", bufs=1, space="PSUM"))
            tlP = ctx.enter_context(tc.tile_pool(name="tail", bufs=2))
            tpsP = ctx.enter_context(tc.tile_pool(name="tps", bufs=1, space="PSUM"))
            ypP = ctx.enter_context(tc.tile_pool(name="yp", bufs=1, space="PSUM"))
            oP = ctx.enter_context(tc.tile_pool(name="outs", bufs=2))
            TW = WT * 128
            for w in range(NWIN):
                esw = esP.tile([12, TW], bf16)
                nc.sync.dma_start(esw[:], esT_ap[:, w * TW:(w + 1) * TW])
                idxw = idP.tile([128, 8 * WT], i16)
                nc.sync.dma_start(idxw[:], idx_ap[:, w * 8 * WT:(w + 1) * 8 * WT])
                ohw = ohP.tile([128, WT * 512], bf16)
                nc.sync.dma_start(ohw[:], oh4_ap[:, w * WT * 512:(w + 1) * WT * 512])
                gt = gP.tile([128, WT, GEL], bf16)
                nc.gpsimd.dma_gather(gt[:], ntab[:], idxw[:], TW, TW, GEL,
                                     single_packet=False)

                hsb = hsP.tile([100, TW], bf16)
                for j in range(TW // 256):
                    # radial MLP borrows the wpp PSUM buffers (free here)
                    hp = wpP.tile([128, PW // 2], f32,
                                  tag="wppA" if j % 2 == 0 else "wppB")
                    nc.tensor.matmul(hp[0:100, 0:256], wr1[:],
                                     esw[:, j * 256:(j + 1) * 256],
                                     start=True, stop=True)
                    nc.scalar.activation(hsb[:, j * 256:(j + 1) * 256],
                                         hp[0:100, 0:256], AF.Silu)

                # acc split in two 320-col tiles (one PSUM bank each):
                # accA = [R_se | R_x], accB = [R_y | R_z]
                accA = accP.tile([128, PW // 2], f32, tag="accA")
                accB = accP.tile([128, PW // 2], f32, tag="accB")
                nc.tensor.matmul(accA[:], ident[:], zt[:, 0:320],
                                 start=True, stop=False)
                nc.tensor.matmul(accB[:], ident[:], zt[:, 0:320],
                                 start=True, stop=False)
                for t in range(WT):
                    wppA = wpP.tile([128, PW // 2], f32, tag="wppA")
                    wppB = wpP.tile([128, PW // 2], f32, tag="wppB")
                    nc.tensor.matmul(wppA[:], hsb[:, t * 128:(t + 1) * 128],
                                     wr2[:, 0:320], start=True, stop=True)
                    nc.tensor.matmul(wppB[:], hsb[:, t * 128:(t + 1) * 128],
                                     wr2[:, 320:640], start=True, stop=True)
                    P = pP.tile([128, PW], bf16)
                    gb = gt[:, t, 0:160].unsqueeze(1).broadcast_to([128, 2, 160])
                    nc.vector.tensor_tensor(
                        P[:, 0:320].rearrange("p (a b) -> p a b", a=2),
                        wppA[:].rearrange("p (a b) -> p a b", a=2),
                        gb, op=MUL)
                    nc.vector.tensor_tensor(
                        P[:, 320:640].rearrange("p (a b) -> p a b", a=2),
                        wppB[:].rearrange("p (a b) -> p a b", a=2),
                        gb, op=MUL)
                    last = t == WT - 1
                    for g in range(4):
                        oh = ohw[:, t * 512 + g * 128:t * 512 + (g + 1) * 128]
                        # g order: x, y, z, se -> P cols 160/320/480/0
                        pc = [160, 320, 480, 0][g]
                        at, r0 = [(accA, 160), (accB, 0), (accB, 160),
                                  (accA, 0)][g]
                        nc.tensor.matmul(at[:, r0:r0 + 160], oh,
                                         P[:, pc:pc + 160],
                                         start=False,
                                         stop=(last and g in (2, 3)))

                # window tail: lin2 + sc + gate
                asb = tlP.tile([128, PW], bf16, tag="asb")
                nc.scalar.activation(asb[:, 0:320], accA[:], AF.Copy)
                nc.scalar.activation(asb[:, 320:640], accB[:], AF.Copy)
                yp = ypP.tile([128, 192], f32)
                for j in range(5):
                    tp = tpsP.tile([128, 128], bf16, tag="tp")
                    nc.tensor.transpose(tp[:], asb[:, j * 128:(j + 1) * 128],
                                        ident[:])
                    ts = tlP.tile([128, 128], bf16, tag="ts")
                    nc.scalar.activation(ts[:], tp[:], AF.Copy)
                    nc.tensor.matmul(yp[:], ts[:], w2p[j][:],
                                     start=(j == 0), stop=(j == 4))
                y2 = tlP.tile([128, 192], f32, tag="y2")
                nc.vector.tensor_tensor(y2[:], yp[:],
                                        scN[:, w * 192:(w + 1) * 192],
                                        op=mybir.AluOpType.add)
                outt = oP.tile([128, 160], f32, tag="outt")
                gtl = oP.tile([128, 32], f32, tag="gtl")
                nc.scalar.activation(outt[:, 0:64], y2[:, 0:64], AF.Silu)
                nc.scalar.activation(gtl[:], y2[:, 64:96], AF.Sigmoid)
                gv = gtl[:].unsqueeze(1).broadcast_to([128, 3, 32])
                nc.vector.tensor_tensor(
                    outt[:, 64:160].rearrange("p (a b) -> p a b", a=3),
                    y2[:, 96:192].rearrange("p (a b) -> p a b", a=3),
                    gv, op=MUL)
                nc.sync.dma_start(out_ap[w * 128:(w + 1) * 128, :], outt[:])

    nc.compile()
    return nc


def kernel(x, z, edge_src, edge_dst, edge_attr, edge_scalars,
           W_sc_s, W_sc_v, W1_s, W1_v, W_r1, W_r2, W2_s, W2_v):
    from concourse import bass_utils
    x = np.asarray(x, np.float32)
    edge_src = np.asarray(edge_src, np.int64)
    edge_dst = np.asarray(edge_dst, np.int64)
    edge_attr = np.asarray(edge_attr, np.float32)
    edge_scalars = np.asarray(edge_scalars, np.float32)

    # uniform tiles-per-window across all cores/windows (SPMD: one program)
    counts = np.zeros((NCORES, NWIN), np.int64)
    cw = (edge_dst // NPC) * NWIN + (edge_dst % NPC) // 128
    u, ct = np.unique(cw, return_counts=True)
    counts.flat[u] = ct
    WT = int(np.ceil(counts.max() / 128.0))
    WT = ((WT + 1) // 2) * 2  # even so TW = WT*128 splits into 256-wide chunks

    key = WT
    if key not in _CACHE:
        _CACHE[key] = _build_program(WT)
    nc = _CACHE[key]

    Wnode, Wsc, Wr1p, Wr2p, W2p = _prep_weights(
        np.asarray(W_sc_s, np.float32), np.asarray(W_sc_v, np.float32),
        np.asarray(W1_s, np.float32), np.asarray(W1_v, np.float32),
        np.asarray(W_r1, np.float32), np.asarray(W_r2, np.float32),
        np.asarray(W2_s, np.float32), np.asarray(W2_v, np.float32))

    in_maps = []
    for c in range(NCORES):
        m = _prep_core(c, x, edge_src, edge_dst, edge_attr, edge_scalars, WT)
        m.update(Wnode=Wnode, Wsc=Wsc, Wr1p=Wr1p, Wr2p=Wr2p, W2p=W2p)
        in_maps.append(m)

    res = bass_utils.run_bass_kernel_spmd(nc, in_maps, core_ids=list(range(NCORES)))
    parts = []
    for c in range(NCORES):
        own_n = min(NPC, N - c * NPC)
        parts.append(res.results[c]["out"][:own_n])
    full = np.concatenate(parts, axis=0)
    out = np.empty((N, 160), np.float32)
    out[:, 0:64] = full[:, 0:64]
    # device gated layout is c-major [32c+u]; reference wants u-major [3u+c]
    out[:, 64:160] = full[:, 64:160].reshape(N, 3, 32).transpose(0, 2, 1).reshape(N, 96)
    return out
